# revision 3
# baseline (speedup 1.0000x reference)
"""Nearest-neighbor retrieval kernel for Trainium2 (8 NeuronCores, SPMD).

Problem: given a query pose [16,96], find argmin_n mean((train_poses[n]-q)^2)
over N=100000 candidates and return target_vels[argmin]  ([25,96]).

Strategy (data parallel over the candidate axis N):
  - Each of the 8 cores gets a 12500-candidate shard of train_poses
    (flattened to [12500, 1536] f32) plus the query broadcast tile.
  - Per 512-candidate tile: one big contiguous DMA (HBM->SBUF, [128, 6144],
    candidate c = base + p*4 + a lives in partition p, free slot a),
    one DVE tensor_sub against the query-broadcast tile (in place), then
    four ACT Square activations with accum_out, which produce the per-
    candidate squared distances directly (sum along the free axis).
  - The [128, 100] per-core distance matrix is DMA'd out; the global
    argmin and the target_vels gather happen on the host (tiny).
  - target_vels never touches the device: only one row of it is needed.
"""

import sys

if "/opt/trn_rl_repo" not in sys.path:
    sys.path.insert(0, "/opt/trn_rl_repo")

import numpy as np

import concourse.bass as bass
import concourse.tile as tile
from concourse import mybir
from concourse.bass_utils import run_bass_kernel_spmd

N = 100000
OBS, PRED, FEAT = 16, 25, 96
K = OBS * FEAT  # 1536
N_CORES = 8
SHARD = N // N_CORES  # 12500
TILE_CANDS = 512
A = TILE_CANDS // 128  # candidates per partition per tile
FREE = A * K  # 6144 f32 per partition per tile

F32 = mybir.dt.float32


def _split_multi_waits(nc, cap: int = 1):
    """The walrus build in this environment supports at most one sync wait
    per instruction; Tile's final drain carries one wait per outstanding
    semaphore lane.  Move excess waits onto fresh same-engine nops inserted
    immediately before the offending instruction (same engine-queue order,
    identical semantics)."""
    for bb in nc.main_func.blocks:
        targets = [
            inst
            for inst in bb.instructions
            if inst.sync_info is not None
            and inst.sync_info.on_wait
            and len(inst.sync_info.on_wait) > cap
        ]
        for inst in targets:
            si = inst.sync_info
            waits = list(si.on_wait)
            extra, keep = waits[:-cap], waits[-cap:]
            nops = []
            for i in range(0, len(extra), cap):
                chunk = extra[i : i + cap]
                nop_inst = nc.engines[inst.engine].nop().ins
                nsi = nop_inst.sync_info
                if nsi is None:
                    nop_inst.sync_info = mybir.SyncInfo(on_wait=chunk, on_update=[])
                else:
                    nsi.on_wait = chunk
                nops.append(nop_inst)
            si.on_wait = keep
            nop_names = {n.name for n in nops}
            for b2 in nc.main_func.blocks:
                cur = list(b2.instructions)
                kept = [i2 for i2 in cur if i2.name not in nop_names]
                if len(kept) != len(cur):
                    b2.instructions = kept
            cur = list(bb.instructions)
            pos = next(i for i, x in enumerate(cur) if x.name == inst.name)
            bb.instructions = cur[:pos] + nops + cur[pos:]


def _bases(shard: int, tile_cands: int) -> list[int]:
    bs = list(range(0, shard - tile_cands + 1, tile_cands))
    if bs[-1] + tile_cands < shard:
        # overlapping final tile: re-computes a few duplicate candidates,
        # which is harmless for argmin (identical values, same indices)
        bs.append(shard - tile_cands)
    return bs


def build_bass(shard: int = SHARD, tile_cands: int = TILE_CANDS):
    a = tile_cands // 128
    free = a * K
    bases = _bases(shard, tile_cands)
    ncols = len(bases) * a

    nc = bass.Bass("TRN2", target_bir_lowering=False, debug=False)
    qb_t = nc.dram_tensor("qb", [128, free], F32, kind="ExternalInput")
    tr_t = nc.dram_tensor("train", [shard, K], F32, kind="ExternalInput")
    d_t = nc.dram_tensor("dists", [128, ncols], F32, kind="ExternalOutput")

    with tile.TileContext(nc) as tc:
        with (
            tc.tile_pool(name="qbp", bufs=1) as qbp,
            tc.tile_pool(name="tp", bufs=4) as tp,
            tc.tile_pool(name="pp", bufs=2, space="PSUM") as pp,
            tc.tile_pool(name="dp", bufs=1) as dp,
        ):
            qb = qbp.tile([128, free], F32)
            nc.sync.dma_start(qb[:], qb_t.ap())

            dists = dp.tile([128, ncols], F32)

            for i, base in enumerate(bases):
                tt = tp.tile([128, free], F32)
                src = tr_t.ap()[base : base + tile_cands, :].rearrange(
                    "(p a) k -> p (a k)", p=128
                )
                nc.sync.dma_start(tt[:], src)
                nc.vector.tensor_sub(tt[:], tt[:], qb[:])
                for j in range(a):
                    ps = pp.tile([128, K], F32)
                    col = i * a + j
                    nc.scalar.activation(
                        ps[:],
                        tt[:, j * K : (j + 1) * K],
                        mybir.ActivationFunctionType.Square,
                        accum_out=dists[:, col : col + 1],
                    )

            nc.sync.dma_start(d_t.ap(), dists[:])

    _split_multi_waits(nc)
    return nc


def make_index_map(shard: int = SHARD, tile_cands: int = TILE_CANDS) -> np.ndarray:
    """idx_map[p, col] = local candidate index whose distance lands there."""
    a = tile_cands // 128
    bases = _bases(shard, tile_cands)
    idx = np.empty((128, len(bases) * a), np.int64)
    p = np.arange(128)
    for i, b in enumerate(bases):
        for j in range(a):
            idx[:, i * a + j] = b + p * a + j
    return idx


_NC_CACHE = {}


def _get_nc(shard: int, tile_cands: int):
    key = (shard, tile_cands)
    if key not in _NC_CACHE:
        _NC_CACHE[key] = build_bass(shard, tile_cands)
    return _NC_CACHE[key]


def kernel(in_pose, train_poses, target_vels, _trace=False):
    in_pose = np.asarray(in_pose)
    train_poses = np.asarray(train_poses)
    target_vels = np.asarray(target_vels)

    q = np.ascontiguousarray(in_pose.reshape(K).astype(np.float32, copy=False))
    qb = np.tile(q[None, :], (128, A))  # [128, FREE]
    train = train_poses.reshape(N, K)

    in_maps = [
        {"qb": qb, "train": train[c * SHARD : (c + 1) * SHARD]}
        for c in range(N_CORES)
    ]

    nc = _get_nc(SHARD, TILE_CANDS)
    res = run_bass_kernel_spmd(nc, in_maps, list(range(N_CORES)), trace=_trace)

    idx_map = make_index_map(SHARD, TILE_CANDS)
    best_val = np.inf
    best_idx = -1
    for c in range(N_CORES):
        d = res.results[c]["dists"]
        v = float(d.min())
        if v > best_val:
            continue
        gi = int((idx_map[d == v] + c * SHARD).min())
        if v < best_val or gi < best_idx:
            best_val, best_idx = v, gi

    out = np.asarray(target_vels[best_idx])
    if _trace:
        return out, res
    return out


# revision 4
# speedup vs baseline: 1.0292x; 1.0292x over previous
"""Nearest-neighbor retrieval kernel for Trainium2 (8 NeuronCores, SPMD).

Problem: given a query pose [16,96], find argmin_n mean((train_poses[n]-q)^2)
over N=100000 candidates and return target_vels[argmin]  ([25,96]).

Strategy (data parallel over the candidate axis N):
  - Host casts train_poses (614 MB f32) to bf16 (307 MB) — the kernel is
    HBM-bandwidth-bound, so halving the bytes halves the runtime.  bf16
    distance noise is ~20 sigma below the top-2 candidate gap, and an exact
    float64 re-check of the device's top-K candidates on the host makes the
    final argmin exact regardless.
  - Each of the 8 cores gets a 12500-candidate shard of train_poses
    (flattened to [12500, 1536]) plus a query broadcast tile.
  - Per 1024-candidate tile: one contiguous DMA (HBM->SBUF [128, 12288],
    candidate c = base + p*8 + a in partition p / free slot a), one DVE
    tensor_sub against the query tile (bf16 runs the DVE 2x perf mode),
    then eight ACT Square activations with accum_out (f32), which emit the
    per-candidate squared distances directly (sum along the free axis).
  - The [128, 104] per-core f32 distance matrix is DMA'd out; the global
    top-K selection + exact recheck + target_vels gather happen on host.
  - target_vels never touches the device: only one row of it is needed.
"""

import sys

if "/opt/trn_rl_repo" not in sys.path:
    sys.path.insert(0, "/opt/trn_rl_repo")

import ml_dtypes
import numpy as np

import concourse.bass as bass
import concourse.tile as tile
from concourse import mybir
from concourse.bass_utils import run_bass_kernel_spmd

N = 100000
OBS, PRED, FEAT = 16, 25, 96
K = OBS * FEAT  # 1536
N_CORES = 8
SHARD = N // N_CORES  # 12500
TILE_CANDS = 1024
RECHECK_K = 256

BF16 = mybir.dt.bfloat16
F32 = mybir.dt.float32
NP_BF16 = ml_dtypes.bfloat16


def _split_multi_waits(nc, cap: int = 1):
    """The walrus build in this environment supports at most one sync wait
    per instruction; Tile's final drain carries one wait per outstanding
    semaphore lane.  Move excess waits onto fresh same-engine nops inserted
    immediately before the offending instruction (same engine-queue order,
    identical semantics)."""
    for bb in nc.main_func.blocks:
        targets = [
            inst
            for inst in bb.instructions
            if inst.sync_info is not None
            and inst.sync_info.on_wait
            and len(inst.sync_info.on_wait) > cap
        ]
        for inst in targets:
            si = inst.sync_info
            waits = list(si.on_wait)
            extra, keep = waits[:-cap], waits[-cap:]
            nops = []
            for i in range(0, len(extra), cap):
                chunk = extra[i : i + cap]
                nop_inst = nc.engines[inst.engine].nop().ins
                nsi = nop_inst.sync_info
                if nsi is None:
                    nop_inst.sync_info = mybir.SyncInfo(on_wait=chunk, on_update=[])
                else:
                    nsi.on_wait = chunk
                nops.append(nop_inst)
            si.on_wait = keep
            nop_names = {n.name for n in nops}
            for b2 in nc.main_func.blocks:
                cur = list(b2.instructions)
                kept = [i2 for i2 in cur if i2.name not in nop_names]
                if len(kept) != len(cur):
                    b2.instructions = kept
            cur = list(bb.instructions)
            pos = next(i for i, x in enumerate(cur) if x.name == inst.name)
            bb.instructions = cur[:pos] + nops + cur[pos:]


def _bases(shard: int, tile_cands: int) -> list[int]:
    bs = list(range(0, shard - tile_cands + 1, tile_cands))
    if bs[-1] + tile_cands < shard:
        # overlapping final tile: re-computes a few duplicate candidates,
        # which is harmless for argmin (identical values, same indices)
        bs.append(shard - tile_cands)
    return bs


def build_bass(shard: int = SHARD, tile_cands: int = TILE_CANDS, dt=BF16):
    a = tile_cands // 128
    free = a * K
    bases = _bases(shard, tile_cands)
    ncols = len(bases) * a

    nc = bass.Bass("TRN2", target_bir_lowering=False, debug=False)
    qb_t = nc.dram_tensor("qb", [128, free], dt, kind="ExternalInput")
    tr_t = nc.dram_tensor("train", [shard, K], dt, kind="ExternalInput")
    d_t = nc.dram_tensor("dists", [128, ncols], F32, kind="ExternalOutput")

    with tile.TileContext(nc) as tc:
        with (
            tc.tile_pool(name="qbp", bufs=1) as qbp,
            tc.tile_pool(name="tp", bufs=4) as tp,
            tc.tile_pool(name="pp", bufs=2, space="PSUM") as pp,
            tc.tile_pool(name="dp", bufs=1) as dp,
        ):
            qb = qbp.tile([128, free], dt)
            nc.sync.dma_start(qb[:], qb_t.ap())

            dists = dp.tile([128, ncols], F32)

            for i, base in enumerate(bases):
                tt = tp.tile([128, free], dt)
                src = tr_t.ap()[base : base + tile_cands, :].rearrange(
                    "(p a) k -> p (a k)", p=128
                )
                nc.sync.dma_start(tt[:], src)
                nc.vector.tensor_sub(tt[:], tt[:], qb[:])
                for j in range(a):
                    ps = pp.tile([128, K], F32)
                    col = i * a + j
                    nc.scalar.activation(
                        ps[:],
                        tt[:, j * K : (j + 1) * K],
                        mybir.ActivationFunctionType.Square,
                        accum_out=dists[:, col : col + 1],
                    )

            nc.sync.dma_start(d_t.ap(), dists[:])

    _split_multi_waits(nc)
    return nc


def make_index_map(shard: int = SHARD, tile_cands: int = TILE_CANDS) -> np.ndarray:
    """idx_map[p, col] = local candidate index whose distance lands there."""
    a = tile_cands // 128
    bases = _bases(shard, tile_cands)
    idx = np.empty((128, len(bases) * a), np.int64)
    p = np.arange(128)
    for i, b in enumerate(bases):
        for j in range(a):
            idx[:, i * a + j] = b + p * a + j
    return idx


_NC_CACHE = {}


def _get_nc(shard: int, tile_cands: int, dt):
    key = (shard, tile_cands, dt)
    if key not in _NC_CACHE:
        _NC_CACHE[key] = build_bass(shard, tile_cands, dt)
    return _NC_CACHE[key]


def kernel(in_pose, train_poses, target_vels, _trace=False):
    in_pose = np.asarray(in_pose)
    train_poses = np.asarray(train_poses)
    target_vels = np.asarray(target_vels)
    a = TILE_CANDS // 128

    q32 = np.ascontiguousarray(in_pose.reshape(K).astype(np.float32, copy=False))
    train32 = train_poses.reshape(N, K)

    q = q32.astype(NP_BF16)
    qb = np.tile(q[None, :], (128, a))  # [128, a*K] bf16
    train = train32.astype(NP_BF16)  # one big host cast, 307 MB

    in_maps = [
        {"qb": qb, "train": train[c * SHARD : (c + 1) * SHARD]}
        for c in range(N_CORES)
    ]

    nc = _get_nc(SHARD, TILE_CANDS, BF16)
    res = run_bass_kernel_spmd(nc, in_maps, list(range(N_CORES)), trace=_trace)

    # global candidate index for every slot of every core's dist matrix
    idx_map = make_index_map(SHARD, TILE_CANDS)  # [128, ncols]
    all_d = np.stack([res.results[c]["dists"] for c in range(N_CORES)])  # [8,128,C]
    all_idx = (
        idx_map[None, :, :] + (np.arange(N_CORES) * SHARD)[:, None, None]
    )  # [8,128,C]

    flat_d = all_d.reshape(-1)
    flat_idx = all_idx.reshape(-1)
    k = min(RECHECK_K, flat_d.size)
    top = np.argpartition(flat_d, k - 1)[:k]
    cands = np.unique(flat_idx[top])

    # exact recheck on host (float64): immune to bf16 distance noise
    diff = train32[cands].astype(np.float64) - q32.astype(np.float64)[None, :]
    exact = (diff * diff).sum(axis=1)
    best_idx = int(cands[np.lexsort((cands, exact))[0]])

    out = np.asarray(target_vels[best_idx])
    if _trace:
        return out, res
    return out
